# revision 34
# baseline (speedup 1.0000x reference)
"""Trainium2 Bass kernel for nn_BasicBlock_90933047591518.

Computation (forward only, STE terms cancel numerically):
    out = BN(conv3x3(sign(x), scale[o] * sign(w)), gamma, beta, mean, var) + x
with scale[o] = mean(|w[o]|).

Key facts used:
  * sign(x), sign(w) are +-1, exactly representable in bf16/fp8e4; the conv
    reduces 128*9 = 1152 such products, so fp32 PSUM accumulation is exact
    (integer magnitudes <= 1152).  The low-precision matmul path is
    therefore *exact*, and the per-channel factor
    scale[o]*gamma[o]*rsqrt(var+eps) folds into one post-conv multiplier.
  * Data parallel: batch N=64 sharded 8 ways (8 images/core); weights/BN
    replicated.  No collectives (inference only).

Per image [C=128 partitions, 56, 56]:
  sign(x) goes into a zero-padded 58x58 grid (flat [128, 3366] + guard
  cols).  Conv output produced in 7 chunks of 8 rows.  Per chunk one PSUM
  bank accumulates the 9 taps:
    - fp8 DoubleRow mode: 4 paired matmuls (taps 2p,2p+1 packed along K
      via overlapping rhs APs) + 1 normal fp8 matmul, free dim 464
      (8 padded rows of 58, garbage edge cols discarded at evacuation).
    - bf16 mode: 9 matmuls with windowed [128, 8, 56] rhs APs.
  Evacuation alternates between ScalarE (activation scale+bias) and
  VectorE (tensor_scalar) per chunk to balance engine load; VectorE adds
  the residual.  Inputs stream on the SP HWDGE queue, outputs on the ACT
  HWDGE queue (two independent FIFOs), both split into half-image
  transfers; the last image stores per chunk to shorten the kernel tail.
"""

import sys
import time

sys.path.insert(0, "/opt/trn_rl_repo")

import numpy as np

import concourse.bacc as bacc
import concourse.tile as tile
from concourse import masks, mybir
from concourse.bass_types import AP
from concourse.bass_utils import run_bass_kernel_spmd

N_CORES = 8
NIMG = 8  # images per core
C = 128
H = W = 56
HP = WP = 58  # padded
RPC = 8  # rows per chunk
NCHUNK = H // RPC  # 7
BN_EPS = 1e-5
USE_FP8 = True

F32 = mybir.dt.float32
BF16 = mybir.dt.bfloat16
FP8 = mybir.dt.float8e4

# tap j = (kh, kw), flat offset in the padded grid
TAP_OFF = [kh * WP + kw for kh in (-1, 0, 1) for kw in (-1, 0, 1)]

_cache = {}


def _build(use_fp8=USE_FP8, xbufs=5, psbufs=6, sign_halves=2, evac_split=True, prepsbufs=2, dma_split=True, fuse_evac=False, abufs=4, obufs=3, pref=3, repeat=1, hw_reps=0, win_rhs=False, w_on_act=False, out_thirds=True, fine_last=False, tail_imgs=1, x0_first=False, sw_il=False):
    nc = bacc.Bacc("TRN2", target_bir_lowering=False, debug=False, num_devices=1)

    xs = nc.dram_tensor("xs", [NIMG, C, H, W], F32, kind="ExternalInput").ap()
    w = nc.dram_tensor("w", [C, C, 3, 3], F32, kind="ExternalInput").ap()
    gamma = nc.dram_tensor("gamma", [C, 1], F32, kind="ExternalInput").ap()
    beta = nc.dram_tensor("beta", [C, 1], F32, kind="ExternalInput").ap()
    bn_mean = nc.dram_tensor("bn_mean", [C, 1], F32, kind="ExternalInput").ap()
    bn_var = nc.dram_tensor("bn_var", [C, 1], F32, kind="ExternalInput").ap()
    out = nc.dram_tensor("out", [NIMG, C, H, W], F32, kind="ExternalOutput").ap()

    with tile.TileContext(nc) as tc:
        _body(nc, tc, xs, w, gamma, beta, bn_mean, bn_var, out, use_fp8, xbufs, psbufs, sign_halves, evac_split, prepsbufs, dma_split, fuse_evac, abufs, obufs, pref, repeat, hw_reps, win_rhs, w_on_act, out_thirds, fine_last, tail_imgs, x0_first, sw_il)

    nc.compile()
    return nc


def _window(t_ap, offset, dims):
    """Hand-built (possibly overlapping) AP on a flat [128, FW] tile view."""
    return AP(
        tensor=t_ap.tensor,
        offset=t_ap.offset + offset,
        ap=[list(t_ap.ap[0])] + [list(d) for d in dims],
    )


def _body(nc, tc, xs, w, gamma, beta, bn_mean, bn_var, out, use_fp8, xbufs=6, psbufs=6, sign_halves=2, evac_split=True, prepsbufs=4, dma_split=True, fuse_evac=False, abufs=3, obufs=2, pref=3, repeat=1, hw_reps=0, win_rhs=False, w_on_act=False, out_thirds=False, fine_last=False, tail_imgs=1, x0_first=False, sw_il=False):
    from contextlib import ExitStack

    adt = FP8 if use_fp8 else BF16
    AFW = HP * WP + 2  # flat a-tile width: lead guard + 58x58 grid + tail guard
    if not use_fp8:
        fuse_evac = False  # bias tap is only emitted on the fp8 path

    if isinstance(dma_split, bool):
        in_split = 2 if dma_split else 1
    else:
        in_split = dma_split

    def dma_in_img(xt, n):
        step = H // in_split
        for h0 in range(0, H, step):
            nc.sync.dma_start(
                xt[:, h0 : h0 + step, :], xs[n, :, h0 : h0 + step, :]
            )

    with ExitStack() as ctx:
        const = ctx.enter_context(tc.tile_pool(name="const", bufs=1))
        w_sign = const.tile([C, 9, C], adt)
        combo_scale = const.tile([C, 1], F32)
        combo_bias = const.tile([C, 1], F32)
        if fuse_evac:
            ones_row = const.tile([C, RPC * WP], BF16)
            cbb_row = const.tile([C, C], BF16)
        if sw_il:
            # DoubleRowSwInterleave weights: per pair p a flat [128, 256] row,
            # flat[2*(127-o)+j] = sign(w)[i, tap 2p+j, o]
            w_sw = const.tile([C, 4, 2 * C], adt)

        xpool = ctx.enter_context(tc.tile_pool(name="x", bufs=xbufs))
        apool = ctx.enter_context(tc.tile_pool(name="a", bufs=abufs))
        opool = ctx.enter_context(tc.tile_pool(name="o", bufs=obufs))
        ypool = ctx.enter_context(tc.tile_pool(name="y", bufs=4))
        pspool = ctx.enter_context(tc.tile_pool(name="ps", bufs=psbufs, space="PSUM"))

        # ---------------- preamble: weight + BN prep ----------------
        with (
            tc.tile_pool(name="pre", bufs=1) as pre,
            tc.tile_pool(name="pre_psum", bufs=prepsbufs, space="PSUM") as pre_psum,
        ):
            # natural-layout weights [o, i, k] (contiguous in DRAM); issue
            # image-0/1 input DMAs right behind it so they overlap the prep
            wo = pre.tile([C, C, 9], F32)
            wdma = nc.scalar.dma_start if w_on_act else nc.sync.dma_start
            if not x0_first:
                wdma(wo[:], w.rearrange("o i kh kw -> o i (kh kw)"))

            xts0 = None
            if hw_reps == 0 and repeat == 1:
                xts0 = []
                for n in range(min(pref, NIMG)):
                    xt = xpool.tile([C, H, W], F32, tag="xt")
                    dma_in_img(xt, n)
                    xts0.append(xt)
                    if x0_first and n == 0:
                        wdma(wo[:], w.rearrange("o i kh kw -> o i (kh kw)"))
            if x0_first and xts0 is None:
                wdma(wo[:], w.rearrange("o i kh kw -> o i (kh kw)"))

            # sign(w) in bf16 (transposed below through the PE)
            ws_o = pre.tile([C, C, 9], BF16)
            nc.scalar.activation(ws_o[:], wo[:], mybir.ActivationFunctionType.Sign)

            ident = pre.tile([C, C], BF16)
            masks.make_identity(nc, ident[:])
            for k in range(9):
                pt = pre_psum.tile([C, C], BF16)
                nc.tensor.transpose(pt[:], ws_o[:, :, k], ident[:])
                nc.vector.tensor_copy(w_sign[:, k, :], pt[:])
            if sw_il:
                for p in range(4):
                    dst = _window(
                        w_sw[:], p * 2 * C + 2 * C - 2, [[1, 2], [-2, C]]
                    )
                    nc.vector.tensor_copy(dst, w_sign[:, 2 * p : 2 * p + 2, :])

            # scale[o] = mean |w[o]| via Abs + accumulate
            wabs = pre.tile([C, C, 9], BF16)
            absacc = pre.tile([C, 1], F32)
            nc.scalar.activation(
                wabs[:], wo[:], mybir.ActivationFunctionType.Abs, accum_out=absacc[:]
            )

            g_sb = pre.tile([C, 1], F32)
            b_sb = pre.tile([C, 1], F32)
            m_sb = pre.tile([C, 1], F32)
            v_sb = pre.tile([C, 1], F32)
            wdma(g_sb[:], gamma)
            wdma(b_sb[:], beta)
            wdma(m_sb[:], bn_mean)
            wdma(v_sb[:], bn_var)

            eps_t = pre.tile([C, 1], F32)
            nc.gpsimd.memset(eps_t[:], BN_EPS)
            sd = pre.tile([C, 1], F32)
            nc.scalar.activation(
                sd[:], v_sb[:], mybir.ActivationFunctionType.Sqrt, bias=eps_t[:]
            )
            inv = pre.tile([C, 1], F32)
            nc.vector.reciprocal(inv[:], sd[:])
            nc.vector.tensor_mul(inv[:], inv[:], g_sb[:])

            nc.scalar.mul(absacc[:], absacc[:], 1.0 / (C * 9))
            nc.vector.tensor_mul(combo_scale[:], absacc[:], inv[:])
            mi = pre.tile([C, 1], F32)
            nc.vector.tensor_mul(mi[:], m_sb[:], inv[:])
            nc.vector.tensor_sub(combo_bias[:], b_sb[:], mi[:])

            if fuse_evac:
                nc.gpsimd.memset(ones_row[:], 1.0)
                rcs = pre.tile([C, 1], F32)
                nc.vector.reciprocal(rcs[:], combo_scale[:])
                cbb = pre.tile([C, 1], BF16)
                nc.vector.tensor_mul(cbb[:], combo_bias[:], rcs[:])
                cpt = pre_psum.tile([C, C], BF16, tag="pt")
                nc.tensor.transpose(cpt[0:1, :], cbb[:], ident[:])
                nc.vector.tensor_copy(cbb_row[0:1, :], cpt[0:1, :])

        # ---------------- main loop over images ----------------
        from contextlib import nullcontext
        PREF = min(pref, NIMG)
        loop_cm = tc.For_i(0, hw_reps, 1) if hw_reps else nullcontext()
        with loop_cm:
         for _rep in range(repeat):
          if xts0 is not None:
              xts = xts0
          else:
              xts = []
              for n in range(PREF):
                  xt = xpool.tile([C, H, W], F32, tag="xt")
                  dma_in_img(xt, n)
                  xts.append(xt)
          for n in range(NIMG):
            xt = xts[n]

            at = apool.tile([C, AFW], adt)
            g = at[:, 1 : 1 + HP * WP].rearrange("p (r c) -> p r c", r=HP)
            # zero padding border + guards (interior fully overwritten by Sign)
            nc.gpsimd.memset(at[:, 0 : WP + 2], 0.0)  # guard + row 0 + (1,0)
            nc.gpsimd.memset(at[:, AFW - WP - 2 : AFW], 0.0)  # (56,57)+row57+guard
            # interior edge pairs (r,57),(r+1,0) for r=1..55
            nc.gpsimd.memset(_window(at[:], 2 * WP, [[WP, HP - 3], [1, 2]]), 0.0)
            # sign in halves so matmuls on early chunks start sooner; the
            # last image signs in chunk-aligned pieces to compress the tail
            if fine_last and n == NIMG - 1:
                pieces = [(0, 9)] + [
                    (RPC * cc + 1, min(RPC * (cc + 1) + 1, H))
                    for cc in range(1, NCHUNK)
                ]
                for lo, hi in pieces:
                    nc.scalar.activation(
                        g[:, lo + 1 : hi + 1, 1 : W + 1],
                        xt[:, lo:hi, :],
                        mybir.ActivationFunctionType.Sign,
                    )
            else:
                hstep = H // sign_halves
                for hh in range(0, H, hstep):
                    nc.scalar.activation(
                        g[:, hh + 1 : hh + hstep + 1, 1 : W + 1],
                        xt[:, hh : hh + hstep, :],
                        mybir.ActivationFunctionType.Sign,
                    )

            ot = opool.tile([C, H, W], F32)
            for c in range(NCHUNK):
                r0 = 1 + RPC * c  # first output row (padded coords)
                ps = pspool.tile(
                    [C, RPC, WP if (use_fp8 and not win_rhs) else W], F32, tag="ps"
                )
                if use_fp8 and win_rhs:
                    # windowed 4D rhs: valid columns only, dense PSUM
                    for p in range(4):
                        base = 2 + r0 * WP + TAP_OFF[2 * p]
                        d = TAP_OFF[2 * p + 1] - TAP_OFF[2 * p]
                        rhs = _window(at[:], base, [[d, 2], [WP, RPC], [1, W]])
                        nc.tensor.matmul(
                            ps[:],
                            w_sign[:, 2 * p : 2 * p + 2, :],
                            rhs,
                            start=(p == 0),
                            stop=False,
                            perf_mode=mybir.MatmulPerfMode.DoubleRow,
                        )
                    base = 2 + r0 * WP + TAP_OFF[8]
                    rhs = _window(at[:], base, [[WP, RPC], [1, W]])
                    nc.tensor.matmul(
                        ps[:], w_sign[:, 8, :], rhs, start=False, stop=not fuse_evac
                    )
                elif use_fp8:
                    # 4 DoubleRow pairs + 1 normal matmul over flat 464 windows
                    for p in range(4):
                        base = 1 + r0 * WP + TAP_OFF[2 * p]
                        d = TAP_OFF[2 * p + 1] - TAP_OFF[2 * p]
                        rhs = _window(at[:], base, [[d, 2], [1, RPC * WP]])
                        if sw_il:
                            lhsT = w_sw[:, p, :].rearrange(
                                "p (two f) -> p two f", two=2
                            )
                            pm = mybir.MatmulPerfMode.DoubleRowSwInterleave
                        else:
                            lhsT = w_sign[:, 2 * p : 2 * p + 2, :]
                            pm = mybir.MatmulPerfMode.DoubleRow
                        nc.tensor.matmul(
                            ps[:],
                            lhsT,
                            rhs,
                            start=(p == 0),
                            stop=False,
                            perf_mode=pm,
                        )
                    base = 1 + r0 * WP + TAP_OFF[8]
                    nc.tensor.matmul(
                        ps[:],
                        w_sign[:, 8, :],
                        at[:, base : base + RPC * WP],
                        start=False,
                        stop=not fuse_evac,
                    )
                else:
                    for j in range(9):
                        kh, kw = j // 3 - 1, j % 3 - 1
                        rhs = g[:, r0 + kh : r0 + kh + RPC, 1 + kw : 1 + kw + W]
                        nc.tensor.matmul(
                            ps[:],
                            w_sign[:, j, :],
                            rhs,
                            start=(j == 0),
                            stop=(j == 8),
                        )
                if use_fp8:
                    if fuse_evac:
                        # bias tap: K=1 matmul of ones row x (cb/cs) row
                        nc.tensor.matmul(
                            ps[:],
                            cbb_row[0:1, :],
                            ones_row[0:1, :],
                            start=False,
                            stop=True,
                        )
                    psv = ps[:] if win_rhs else ps[:, :, 1 : 1 + W]
                else:
                    psv = ps[:]

                if fuse_evac:
                    rows = slice(RPC * c, RPC * (c + 1))
                    if n >= NIMG - tail_imgs:
                        zt = ypool.tile([C, RPC, W], F32, tag="zt")
                        nc.vector.scalar_tensor_tensor(
                            zt[:], psv, combo_scale[:], xt[:, rows, :],
                            mybir.AluOpType.mult, mybir.AluOpType.add,
                        )
                        nc.scalar.dma_start(out[n, :, rows, :], zt[:])
                    else:
                        nc.vector.scalar_tensor_tensor(
                            ot[:, rows, :], psv, combo_scale[:], xt[:, rows, :],
                            mybir.AluOpType.mult, mybir.AluOpType.add,
                        )
                        if out_thirds:
                            if c == 1:
                                nc.scalar.dma_start(out[n, :, : 2 * RPC, :], ot[:, : 2 * RPC, :])
                            elif c == 3:
                                nc.scalar.dma_start(out[n, :, 2 * RPC : 4 * RPC, :], ot[:, 2 * RPC : 4 * RPC, :])
                            elif c == NCHUNK - 1:
                                nc.scalar.dma_start(out[n, :, 4 * RPC :, :], ot[:, 4 * RPC :, :])
                        elif dma_split and c == 2:
                            nc.scalar.dma_start(out[n, :, : 3 * RPC, :], ot[:, : 3 * RPC, :])
                        elif dma_split and c == NCHUNK - 1:
                            nc.scalar.dma_start(out[n, :, 3 * RPC :, :], ot[:, 3 * RPC :, :])
                    continue
                yt = ypool.tile([C, RPC, W], F32)
                if (not evac_split) or c % 2 == 0:
                    nc.scalar.activation(
                        yt[:],
                        psv,
                        mybir.ActivationFunctionType.Identity,
                        bias=combo_bias[:],
                        scale=combo_scale[:],
                    )
                else:
                    nc.vector.tensor_scalar(
                        yt[:],
                        psv,
                        combo_scale[:],
                        combo_bias[:],
                        mybir.AluOpType.mult,
                        mybir.AluOpType.add,
                    )
                rows = slice(RPC * c, RPC * (c + 1))
                if n >= NIMG - tail_imgs:
                    # trailing images: store per chunk to shorten the tail
                    zt = ypool.tile([C, RPC, W], F32, tag="zt")
                    nc.vector.tensor_add(zt[:], yt[:], xt[:, rows, :])
                    nc.scalar.dma_start(out[n, :, rows, :], zt[:])
                else:
                    nc.vector.tensor_add(ot[:, rows, :], yt[:], xt[:, rows, :])
                    if out_thirds:
                        if c == 1:
                            nc.scalar.dma_start(out[n, :, : 2 * RPC, :], ot[:, : 2 * RPC, :])
                        elif c == 3:
                            nc.scalar.dma_start(out[n, :, 2 * RPC : 4 * RPC, :], ot[:, 2 * RPC : 4 * RPC, :])
                        elif c == NCHUNK - 1:
                            nc.scalar.dma_start(out[n, :, 4 * RPC :, :], ot[:, 4 * RPC :, :])
                    elif dma_split and c == 2:
                        nc.scalar.dma_start(out[n, :, : 3 * RPC, :], ot[:, : 3 * RPC, :])
                    elif dma_split and c == NCHUNK - 1:
                        nc.scalar.dma_start(out[n, :, 3 * RPC :, :], ot[:, 3 * RPC :, :])

            if n < NIMG - tail_imgs and not dma_split:
                nc.scalar.dma_start(out[n], ot[:])
            if n + pref < NIMG:
                xt2 = xpool.tile([C, H, W], F32, tag="xt")
                dma_in_img(xt2, n + pref)
                xts.append(xt2)


def kernel(x, weight, gamma, beta, bn_mean, bn_var):
    if "nc" not in _cache:
        _cache["nc"] = _build()
    nc = _cache["nc"]

    x = np.ascontiguousarray(x, dtype=np.float32)
    per = x.shape[0] // N_CORES
    rep = {
        "w": np.ascontiguousarray(weight, dtype=np.float32),
        "gamma": np.ascontiguousarray(gamma, dtype=np.float32).reshape(C, 1),
        "beta": np.ascontiguousarray(beta, dtype=np.float32).reshape(C, 1),
        "bn_mean": np.ascontiguousarray(bn_mean, dtype=np.float32).reshape(C, 1),
        "bn_var": np.ascontiguousarray(bn_var, dtype=np.float32).reshape(C, 1),
    }
    in_maps = [
        {"xs": x[c * per : (c + 1) * per], **rep} for c in range(N_CORES)
    ]
    res = run_bass_kernel_spmd(nc, in_maps, core_ids=list(range(N_CORES)))
    return np.concatenate([res.results[c]["out"] for c in range(N_CORES)], axis=0)


if __name__ == "__main__":
    t0 = time.time()
    _cache["nc"] = _build()
    print("build+compile:", time.time() - t0)
